# revision 4
# baseline (speedup 1.0000x reference)
"""Bass/Tile kernel v3: sparse sliding-window attention with sinks.

Problem (full): B=4, N=1024, DIM=1024, H=16, D=64, SW=256.
Sharding: 8 cores; core c -> batch b=c//2, head-group g=c%2 (8 heads each).
Host sums the two per-head-group partial projections + proj bias.

v3 design vs v2:
  - scores computed TRANSPOSED: sc^T[k, h, q] via (k^T stationary, q^T
    moving) so exp writes p^T directly -- the per-qi 768KB XBAR transpose
    of probabilities is gone.
  - sliding-window masks injected on the PE as one extra accumulate
    matmul per masked key-block (identity stationary, replicated mask
    moving) -- no DVE touches score PSUM.
  - PV uses p^T as stationary and [v | ones] as moving: output lands
    q-major and the softmax denominator drops out free in column 64,
    so normalization is plain [P,1]-broadcast DVE ops.
  - attention output transposed back for the projection by one small
    (128KB) XBAR DMA per qi; q/k transposes moved from PE to XBAR DMA
    (both transpose streams share the sync HWDGE FIFO -> serialized).
  - rstd = exp(-0.5*ln(ms+eps)) fused across q+k: one Ln + one Exp on
    [P,16] per tile instead of four activations.
  - software pipeline: step t interleaves scores/exp(t-2) block-by-block
    between the QKV chunk matmuls of tile t, then PV/norm(t-3) and
    proj(t-4), so the PE never waits on the exp ping-pong.
"""

import sys

sys.path.insert(0, "/opt/trn_rl_repo")

import numpy as np
import ml_dtypes

import concourse.bass as bass
import concourse.mybir as mybir
import concourse.tile as tile
from concourse import bacc

# Pin all activations to the one table that holds every func we use
# (Square, Ln, Exp, Copy) so the compiled kernel performs a single
# LoadActFuncSet instead of thrashing between per-func tables.
_ACT_SET = "natural_log_exp_and_others"
_real_gat = None


def _gat_pinned(arch):
    tabs = _real_gat(arch)
    return {name: (funcs if name == _ACT_SET else set())
            for name, funcs in tabs.items()}

F32 = mybir.dt.float32
BF16 = mybir.dt.bfloat16

B, N, DIM = 4, 1024, 1024
H, D = 16, 64
SW = 256
ROPE_BASE = 10000.0
LN_EPS = 1e-5
P = 128
NT = N // P      # 8 query/n tiles
CC = DIM // P    # 8 contraction chunks
HL = H // 2      # 8 local heads
NEG = -1.0e30


def build_nc(repeat=1, use_for_i=False):
    global _real_gat
    if _real_gat is None:
        _real_gat = bacc.get_activation_tables
    bacc.get_activation_tables = _gat_pinned
    try:
        return _build_nc_inner(repeat, use_for_i)
    finally:
        bacc.get_activation_tables = _real_gat


def _build_nc_inner(repeat, use_for_i):
    nc = bacc.Bacc("TRN2", target_bir_lowering=False, debug=False, num_devices=8)

    xt = nc.declare_dram_parameter("xt", [DIM, N], BF16, isOutput=False)
    wqkt = nc.declare_dram_parameter("wqkt", [DIM, 1024], BF16, isOutput=False)
    wvt = nc.declare_dram_parameter("wvt", [DIM, 512], BF16, isOutput=False)
    projt = nc.declare_dram_parameter("projt", [512, DIM], BF16, isOutput=False)
    cos2 = nc.declare_dram_parameter("cos2", [N, 2 * D], BF16, isOutput=False)
    sin2 = nc.declare_dram_parameter("sin2", [N, 2 * D], BF16, isOutput=False)
    esink = nc.declare_dram_parameter("esink", [1, HL], F32, isOutput=False)
    masksT = nc.declare_dram_parameter("masksT", [P, 2 * HL * P], BF16,
                                       isOutput=False)
    identb = nc.declare_dram_parameter("identb", [P, P], BF16, isOutput=False)
    y = nc.declare_dram_parameter("y", [N, DIM], F32, isOutput=True)

    with tile.TileContext(nc) as tc:
        with tc.tile_pool(name="consts", bufs=1) as consts:
            wqk_sb = consts.tile([P, CC, 1024], BF16, tag="wqk")
            wqk_src = wqkt.ap().rearrange("(cc p) f -> p cc f", p=P)
            wv_sb = consts.tile([P, CC, 512], BF16, tag="wv")
            wv_src = wvt.ap().rearrange("(cc p) f -> p cc f", p=P)
            for c in range(CC):
                nc.sync.dma_start(out=wqk_sb[:, c, :], in_=wqk_src[:, c, :])
                nc.sync.dma_start(out=wv_sb[:, c, :], in_=wv_src[:, c, :])
            pj_sb = consts.tile([P, 4, DIM], BF16, tag="pj")
            pj_src = projt.ap().rearrange("(ch p) e -> p ch e", p=P)
            for ch in range(4):
                nc.sync.dma_start(out=pj_sb[:, ch, :], in_=pj_src[:, ch, :])
            cos_sb = consts.tile([P, NT, 2, D], BF16, tag="cos")
            nc.sync.dma_start(
                out=cos_sb, in_=cos2.ap().rearrange("(t p) (s d) -> p t s d", p=P, d=D))
            sin_sb = consts.tile([P, NT, 2, D], BF16, tag="sin")
            nc.sync.dma_start(
                out=sin_sb, in_=sin2.ap().rearrange("(t p) (s d) -> p t s d", p=P, d=D))
            es_sb = consts.tile([P, HL], F32, tag="es")
            nc.sync.dma_start(out=es_sb, in_=esink.ap().to_broadcast([P, HL]))
            mk_sb = consts.tile([P, 2, HL, P], BF16, tag="mk")
            nc.sync.dma_start(out=mk_sb, in_=masksT.ap().rearrange(
                "p (ty h c) -> p ty h c", h=HL, c=P))
            idb_sb = consts.tile([P, P], BF16, tag="idb")
            nc.sync.dma_start(out=idb_sb, in_=identb.ap())
            eps_sb = consts.tile([P, 1], F32, tag="eps")
            nc.vector.memset(eps_sb, LN_EPS)

            # persistent intermediates
            qkt_sb = consts.tile([P, CC, N], BF16, tag="qkt")  # [f%128, f//128, n]
            vext = consts.tile([P, NT, HL, 65], BF16, tag="vext")
            nc.vector.memset(vext[:, :, :, 64:65], 1.0)  # denominator ones col
            sq_g = consts.tile([P, NT, 16], F32, tag="sqg")
            rstd_g = consts.tile([P, NT, 16], F32, tag="rstdg")

            def body():
                with (
                    tc.tile_pool(name="pA", bufs=2) as pA,
                    tc.tile_pool(name="psA", bufs=1, space="PSUM") as psA,
                    tc.tile_pool(name="pB", bufs=2) as pB,
                    tc.tile_pool(name="psB", bufs=1, space="PSUM") as psB,
                ):
                    xt_src = xt.ap().rearrange("(cc p) n -> p cc n", p=P)
                    xt_hist, ps_hist, z_hist, pexp_hist = {}, {}, {}, {}
                    at_hist, attT_hist, scT_hist = {}, {}, {}

                    def prefetch(t):
                        xt_t = pA.tile([P, CC, P], BF16, tag="xt", name=f"xt{t}")
                        nc.sync.dma_start(out=xt_t,
                                          in_=xt_src[:, :, t * P:(t + 1) * P])
                        xt_hist[t] = xt_t

                    def qkv_half(t, half):
                        xt_t = xt_hist[t]
                        if half == 0:
                            ps_q = psA.tile([P, 512], F32, tag="psq",
                                            name=f"psq{t}")
                            ps_k = psA.tile([P, 512], F32, tag="psk",
                                            name=f"psk{t}")
                            ps_v = psA.tile([P, 512], F32, tag="psv",
                                            name=f"psv{t}")
                            ps_hist[t] = (ps_q, ps_k, ps_v)
                        ps_h = ps_hist[t][half]
                        for c in range(CC):
                            nc.tensor.matmul(
                                ps_h, xt_t[:, c, :],
                                wqk_sb[:, c, half * 512:half * 512 + 512],
                                start=(c == 0), stop=(c == CC - 1))
                        hsl = slice(half * 8, half * 8 + 8)
                        scr = pA.tile([P, 512], BF16, tag=f"scr{half}",
                                      name=f"scr{t}_{half}")
                        nc.scalar.activation(
                            out=scr, in_=ps_h,
                            func=mybir.ActivationFunctionType.Square)
                        nc.vector.tensor_reduce(
                            out=sq_g[:, t, hsl],
                            in_=scr.rearrange("p (h d) -> p h d", d=D),
                            axis=mybir.AxisListType.X, op=mybir.AluOpType.add)
                        if half == 1:
                            # rstd for q AND k heads in one Ln + one Exp
                            nc.scalar.activation(
                                out=rstd_g[:, t, :], in_=sq_g[:, t, :],
                                func=mybir.ActivationFunctionType.Ln,
                                scale=1.0 / D, bias=eps_sb)
                            nc.scalar.activation(
                                out=rstd_g[:, t, :], in_=rstd_g[:, t, :],
                                func=mybir.ActivationFunctionType.Exp,
                                scale=-0.5)

                    def qkv_v(t):
                        xt_t = xt_hist.pop(t)
                        ps_v = ps_hist[t][2]
                        for c in range(CC):
                            nc.tensor.matmul(ps_v, xt_t[:, c, :], wv_sb[:, c, :],
                                             start=(c == 0), stop=(c == CC - 1))
                        nc.scalar.copy(
                            out=vext[:, t, :, 0:64],
                            in_=ps_v.rearrange("p (h d) -> p h d", d=D))

                    def rope_tp(t):
                        ps_q, ps_k, _ = ps_hist.pop(t)
                        zbt = pA.tile([P, 1024], BF16, tag="zb", name=f"zb{t}")
                        for half, ps_h in ((0, ps_q), (1, ps_k)):
                            hsl = slice(half * 8, half * 8 + 8)
                            rb = rstd_g[:, t, hsl].unsqueeze(2)\
                                .broadcast_to([P, 8, D])
                            z3 = zbt[:, half * 512:half * 512 + 512]\
                                .rearrange("p (h d) -> p h d", d=D)
                            nc.vector.tensor_mul(
                                out=z3,
                                in0=ps_h.rearrange("p (h d) -> p h d", d=D),
                                in1=rb)
                        # RoPE; rotate-half via sliced views
                        t3 = pA.tile([P, 1024], BF16, tag="t3", name=f"t3{t}")
                        r3 = pA.tile([P, 1024], BF16, tag="r3", name=f"r3{t}")
                        zf = pA.tile([P, 1024], BF16, tag="zf", name=f"zf{t}")
                        z4 = zbt.rearrange("p (s h d) -> p s h d", s=2, d=D)
                        t4 = t3.rearrange("p (s h d) -> p s h d", s=2, d=D)
                        r4 = r3.rearrange("p (s h d) -> p s h d", s=2, d=D)
                        cb = cos_sb[:, t].unsqueeze(2).broadcast_to([P, 2, HL, D])
                        sb_ = sin_sb[:, t].unsqueeze(2).broadcast_to([P, 2, HL, D])
                        J = D // 2
                        nc.vector.tensor_mul(out=t4, in0=z4, in1=cb)
                        nc.gpsimd.tensor_tensor(
                            out=r4[:, :, :, 0:J], in0=z4[:, :, :, J:D],
                            in1=sb_[:, :, :, 0:J], op=mybir.AluOpType.mult)
                        nc.gpsimd.tensor_tensor(
                            out=r4[:, :, :, J:D], in0=z4[:, :, :, 0:J],
                            in1=sb_[:, :, :, J:D], op=mybir.AluOpType.mult)
                        nc.vector.tensor_add(out=zf, in0=t3, in1=r3)
                        # q^T/k^T into qkt via the XBAR transpose engine:
                        # qkt[p, c, t*128+r] = zf[r, c*128+p]
                        nc.sync.dma_start_transpose(
                            out=qkt_sb[:, :, t * P:(t + 1) * P], in_=zf)

                    def scores_blk(qi, b):
                        kb0 = max(qi - 2, 0)
                        nkb = qi - kb0 + 1
                        if b >= nkb:
                            return
                        kb = kb0 + b
                        qsl = slice(qi * P, (qi + 1) * P)
                        ksl = slice(kb * P, (kb + 1) * P)
                        if b == 0:
                            scT = psB.tile([P, HL, P], F32, tag="scT",
                                           name=f"scT{qi}")
                            scT_hist[qi] = scT
                            pexp = pB.tile([P, 3, HL, P], BF16, tag="pexp",
                                           name=f"pexp{qi}")
                            pexp_hist[qi] = pexp
                        scT, pexp = scT_hist[qi], pexp_hist[qi]
                        # mask type: 0 = far block (NEG where k<=q),
                        # 1 = diag block (NEG where k>q), none for mid block
                        mty = 0 if kb == qi - 2 else (1 if kb == qi else None)
                        # per 2KB-psum-bank group (4 heads): optional mask
                        # accumulate (identity stationary) first, score
                        # matmuls pile on, last one closes the group
                        for g4 in range(2):
                            hs = slice(4 * g4, 4 * g4 + 4)
                            if mty is not None:
                                nc.tensor.matmul(
                                    scT[:, hs].rearrange("p h q -> p (h q)"),
                                    idb_sb,
                                    mk_sb[:, mty, hs].rearrange(
                                        "p h c -> p (h c)"),
                                    start=True, stop=False)
                            for hh in range(4):
                                h = 4 * g4 + hh
                                poff = (h % 2) * 64
                                nc.tensor.matmul(
                                    scT[:, h, :],
                                    qkt_sb[poff:poff + 64, 4 + h // 2, ksl],
                                    qkt_sb[poff:poff + 64, h // 2, qsl],
                                    start=(mty is None and hh == 0),
                                    stop=(hh == 3))
                        nc.scalar.activation(
                            out=pexp[:, b], in_=scT,
                            func=mybir.ActivationFunctionType.Exp, scale=0.125)
                        if b == nkb - 1:
                            scT_hist.pop(qi)

                    def pv_norm(qi):
                        kb0 = max(qi - 2, 0)
                        nkb = qi - kb0 + 1
                        pexp = pexp_hist.pop(qi)
                        at_lo = psB.tile([P, 4, 65], F32, tag="bps", bufs=2,
                                         name=f"atlo{qi}")
                        at_hi = psB.tile([P, 4, 65], F32, tag="bps", bufs=2,
                                         name=f"athi{qi}")
                        for h in range(HL):
                            at = at_lo if h < 4 else at_hi
                            for b in range(nkb):
                                nc.tensor.matmul(
                                    at[:, h % 4, :],
                                    pexp[:, b, h, :],
                                    vext[:, kb0 + b, h, :],
                                    start=(b == 0), stop=(b == nkb - 1))
                        den8 = pB.tile([P, HL], F32, tag="den8",
                                       name=f"den{qi}")
                        rec8 = pB.tile([P, HL], F32, tag="rec8",
                                       name=f"rec{qi}")
                        att4 = pB.tile([P, HL, 64], BF16, tag="att4",
                                       name=f"att4_{qi}")
                        nc.vector.tensor_add(out=den8[:, 0:4],
                                             in0=at_lo[:, :, 64],
                                             in1=es_sb[:, 0:4])
                        nc.vector.tensor_add(out=den8[:, 4:8],
                                             in0=at_hi[:, :, 64],
                                             in1=es_sb[:, 4:8])
                        nc.vector.reciprocal_approx_fast(out=rec8, in_=den8)
                        nc.vector.tensor_mul(
                            out=att4[:, 0:4, :], in0=at_lo[:, :, 0:64],
                            in1=rec8[:, 0:4].unsqueeze(2)
                                .broadcast_to([P, 4, 64]))
                        nc.vector.tensor_mul(
                            out=att4[:, 4:8, :], in0=at_hi[:, :, 0:64],
                            in1=rec8[:, 4:8].unsqueeze(2)
                                .broadcast_to([P, 4, 64]))
                        attT = pB.tile([P, 4, P], BF16, tag="attT",
                                       name=f"attT{qi}")
                        nc.sync.dma_start_transpose(
                            out=attT, in_=att4.rearrange("p h d -> p (h d)"))
                        attT_hist[qi] = attT

                    def proj_y(qi):
                        attT = attT_hist.pop(qi)
                        y_sb = pB.tile([P, 1024], F32, tag="ysb",
                                       name=f"ysb{qi}")
                        for e in range(2):
                            pj_ps = psB.tile([P, 4, P], F32, tag="bps",
                                             bufs=2, name=f"pjp{qi}_{e}")
                            pj_v = pj_ps.rearrange("p a b -> p (a b)")
                            for ch in range(4):
                                nc.tensor.matmul(
                                    pj_v,
                                    attT[:, ch, :],
                                    pj_sb[:, ch, e * 512:(e + 1) * 512],
                                    start=(ch == 0), stop=(ch == 3))
                            if e == 0:
                                nc.scalar.copy(out=y_sb[:, 0:512], in_=pj_v)
                            else:
                                nc.vector.tensor_copy(out=y_sb[:, 512:1024],
                                                      in_=pj_v)
                        nc.scalar.dma_start(out=y[qi * P:(qi + 1) * P, :],
                                            in_=y_sb)

                    # software pipeline: step t runs scores+exp(t-2)
                    # interleaved into QKV(t)'s matmul chunks, then
                    # PV/norm(t-3) and proj(t-4).
                    prefetch(0)
                    for t in range(NT + 4):
                        if t + 1 < NT:
                            prefetch(t + 1)
                        qs, qp, qj = t - 2, t - 3, t - 4
                        if 0 <= qs < NT:
                            scores_blk(qs, 0)
                        if t < NT:
                            qkv_half(t, 0)
                        if 0 <= qs < NT:
                            scores_blk(qs, 1)
                        if t < NT:
                            qkv_half(t, 1)
                        if 0 <= qs < NT:
                            scores_blk(qs, 2)
                        if t < NT:
                            qkv_v(t)
                            rope_tp(t)
                        if 0 <= qp < NT:
                            pv_norm(qp)
                        if 0 <= qj < NT:
                            proj_y(qj)

            if use_for_i and repeat > 1:
                with tc.For_i(0, repeat, 1):
                    body()
            else:
                for _ in range(repeat):
                    body()

    nc.finalize()
    return nc


def host_prep(x, qkv_w, qn_w, qn_b, kn_w, kn_b, sinks, proj_w, proj_b):
    """Build the 8 per-core input maps (numpy, host-side sharding + tables)."""
    f32 = np.float32
    bf16 = ml_dtypes.bfloat16
    n = np.arange(N, dtype=np.float64)
    inv = ROPE_BASE ** (-np.arange(0, D, 2, dtype=np.float64) / D)
    freqs = n[:, None] * inv[None, :]
    emb = np.concatenate([freqs, freqs], axis=1)
    cos, sin = np.cos(emb), np.sin(emb)
    sgn = np.concatenate([-np.ones(D // 2), np.ones(D // 2)])

    def tables(w):
        w = np.asarray(w, np.float64)
        w_rot = np.concatenate([w[D // 2:], w[:D // 2]])
        cw = cos * w[None, :]
        sw = sin * w_rot[None, :] * sgn[None, :]
        return cw, sw

    cwq, swq = tables(qn_w)
    cwk, swk = tables(kn_w)
    assert np.allclose(qn_b, 0) and np.allclose(kn_b, 0), \
        "nonzero qk-norm bias not implemented"
    # [N, 2, D] -> [N, 2*D], slot 0 = q tables, slot 1 = k tables
    cos2_np = np.ascontiguousarray(
        np.stack([cwq, cwk], axis=1).reshape(N, 2 * D).astype(bf16))
    sin2_np = np.ascontiguousarray(
        np.stack([swq, swk], axis=1).reshape(N, 2 * D).astype(bf16))

    # transposed masks, [k-row, q-col], replicated per head:
    # type 0 (far block kb=qi-2): NEG where kr <= qr
    # type 1 (diag block kb=qi):  NEG where kr >  qr
    r = np.arange(P)[:, None]
    c = np.arange(P)[None, :]
    m0 = np.where(r <= c, NEG, 0.0)
    m1 = np.where(r > c, NEG, 0.0)
    mrep = np.stack(
        [np.broadcast_to(m0[:, None, :], (P, HL, P)),
         np.broadcast_to(m1[:, None, :], (P, HL, P))], axis=1)
    masksT_np = np.ascontiguousarray(mrep.reshape(P, 2 * HL * P).astype(bf16))

    def center(wh):
        # wh: [512, DIM] rows grouped per head (64 rows each); subtract
        # per-head row-mean so z' = z - mean_d(z) exactly (mean linear in x)
        w3 = np.asarray(wh, np.float64).reshape(8, D, DIM)
        w3 = w3 - w3.mean(axis=1, keepdims=True)
        return w3.reshape(512, DIM)

    in_maps = []
    for core in range(8):
        b, g = core // 2, core % 2
        q_rows = center(qkv_w[g * 512:(g + 1) * 512])
        k_rows = center(qkv_w[1024 + g * 512:1024 + (g + 1) * 512])
        v_rows = qkv_w[2048 + g * 512:2048 + (g + 1) * 512]
        in_maps.append({
            "xt": np.ascontiguousarray(x[b].T.astype(bf16)),
            "wqkt": np.ascontiguousarray(
                np.concatenate([q_rows, k_rows], 0).T.astype(bf16)),
            "wvt": np.ascontiguousarray(np.asarray(v_rows).T.astype(bf16)),
            "projt": np.ascontiguousarray(
                proj_w[:, g * 512:(g + 1) * 512].T.astype(bf16)),
            "cos2": cos2_np, "sin2": sin2_np,
            "esink": np.exp(np.asarray(sinks[g * 8:(g + 1) * 8], np.float64))
                .astype(f32).reshape(1, HL),
            "masksT": masksT_np,
            "identb": np.eye(P).astype(bf16),
        })
    return in_maps


def assemble(results, proj_b):
    out = np.zeros((B, N, DIM), dtype=np.float32)
    for b in range(B):
        out[b] = results[2 * b]["y"] + results[2 * b + 1]["y"] + proj_b[None, :]
    return out


from concourse.bass_utils import run_bass_kernel_spmd

_NC_CACHE = {}


def _get_nc():
    if "nc" not in _NC_CACHE:
        _NC_CACHE["nc"] = build_nc(repeat=1)
    return _NC_CACHE["nc"]


def kernel(x, qkv_w, qn_w, qn_b, kn_w, kn_b, sinks, proj_w, proj_b):
    x = np.asarray(x, np.float32)
    qkv_w = np.asarray(qkv_w, np.float32)
    proj_w = np.asarray(proj_w, np.float32)
    in_maps = host_prep(x, qkv_w, np.asarray(qn_w), np.asarray(qn_b),
                        np.asarray(kn_w), np.asarray(kn_b),
                        np.asarray(sinks), proj_w, np.asarray(proj_b))
    nc = _get_nc()
    res = run_bass_kernel_spmd(nc, in_maps, core_ids=list(range(8)))
    return assemble(res.results, np.asarray(proj_b, np.float32))
